# revision 3
# baseline (speedup 1.0000x reference)
"""Trainium2 Bass kernel for the sparse-attention scores module.

Computes, for each batch b:
    scores[b, :] = softmax_s( v . tanh(W1 @ static[b] + W2 @ dynamic[b] + W3 @ hidden[b]) )
with W = [W1 | W2 | W3] of shape [H, 3H], static/dynamic [B, H, S], hidden [B, H].

Sharding: data-parallel over B across 8 NeuronCores (8 batches per core).

The encoder streams are quantized to fp8 (e4m3) on the host, which halves the
HBM traffic vs bf16 and enables the PE's DoubleRow mode (0.5 cycles/row,
256-deep contraction per instruction). The [H,3H] weight is quantized to fp8
as a (Wq, Wr) pair — Wr is the fp8 quantization residual at the same
fixed-point scale — so accumulating both matmuls in PSUM removes the weight
quantization error entirely and only the encoder quantization error
(~1.4e-2 rel l2, measured) remains. Scales: x*8, W*128, undone by the ACT
engine's pre-tanh scale = 1/1024.

The v-dot runs in bf16 on the PE. Each (batch, chunk) v-dot uses a one-hot
weight tile (v in column 8*b+n, zeros elsewhere) so all 64 results accumulate
directly into one [64, 512] PSUM bank at the right partition — no per-chunk
staging copies or SBUF->SBUF DMAs. The softmax epilogue reads that bank.
"""

import sys

sys.path.insert(0, "/opt/trn_rl_repo")

import numpy as np
import ml_dtypes

B, H, S = 64, 256, 4096
N_CORES = 8
BPC = B // N_CORES          # batches per core
KK = H // 128               # 2 contraction chunks of the output dim
NCH = S // 512              # 8 psum column chunks
NQ = 4                      # input DMA quarters along s
SQ = S // NQ                # 1024 columns per quarter
NROW = BPC * NCH            # 64 score chunks = psum partitions

X_SCALE = 8.0
W_SCALE = 128.0
INV_SCALE = 1.0 / (X_SCALE * W_SCALE)

FP8 = ml_dtypes.float8_e4m3
BF16 = ml_dtypes.bfloat16


def build_bass(reps: int = 1, loop_iters: int = 0):
    """Build the per-core Bass program. reps>1 unrolls the whole computation
    multiple times; loop_iters>0 additionally wraps the unrolled body in a
    hardware loop. Both are used only for timing by differencing."""
    import contextlib

    import concourse.bacc as bacc
    import concourse.tile as tile
    from concourse import mybir

    f32 = mybir.dt.float32
    f32r = mybir.dt.float32r
    bf16 = mybir.dt.bfloat16
    fp8 = mybir.dt.float8e4
    DR = mybir.MatmulPerfMode.DoubleRow

    nc = bacc.Bacc(None)

    # Encoder data, fp8, packed as [b, q, p, t, i, sq]:
    #   element = x_t[b, h = i*128 + p, s = q*SQ + sq] * X_SCALE
    # so each (b, q) transfer is fully contiguous with 4 KiB per partition.
    xq = nc.dram_tensor("xq", [BPC, NQ, 128, 2, 2, SQ], fp8, kind="ExternalInput")
    # fp8 weights [p, t, r, i, m, mc]: lhsT tiles for DoubleRow;
    # r=0 is q8(W*W_SCALE), r=1 is the quantization residual (same scale).
    wq = nc.dram_tensor("wq", [128, 2, 2, 2, 2, 128], fp8, kind="ExternalInput")
    # W3 tiles (f32) for the per-batch bias, layout [p, kk*2+m, mc].
    w3 = nc.dram_tensor("w3", [128, 4, 128], f32r, kind="ExternalInput")
    ht = nc.dram_tensor("ht", [128, KK, BPC], f32r, kind="ExternalInput")
    # One-hot v tiles [p, row, m, c]: vh[p, row, m, c] = v[m*128+p] * (c==row).
    vh = nc.dram_tensor("vh", [128, NROW, KK, NROW], bf16, kind="ExternalInput")
    out = nc.dram_tensor("out", [BPC, S], f32, kind="ExternalOutput")

    with tile.TileContext(nc) as tc:
        with (
            tc.tile_pool(name="consts", bufs=1) as consts,
            tc.tile_pool(name="xpool", bufs=3) as xpool,
            tc.tile_pool(name="tpool", bufs=6) as tpool,
            tc.tile_pool(name="spool", bufs=2) as spool,
            tc.tile_pool(name="mpsum", bufs=4, space="PSUM") as mpsum,
            tc.tile_pool(name="vpsum", bufs=2, space="PSUM") as vpsum,
            tc.tile_pool(name="spsum", bufs=2, space="PSUM") as spsum,
        ):
            wq_sb = consts.tile([128, 2, 2, 2, 2, 128], fp8)
            nc.sync.dma_start(out=wq_sb, in_=wq[:, :, :, :, :, :])
            w3_sb = consts.tile([128, 4, 128], f32r)
            nc.sync.dma_start(out=w3_sb, in_=w3[:, :, :])
            ht_sb = consts.tile([128, KK, BPC], f32r)
            nc.sync.dma_start(out=ht_sb, in_=ht[:, :, :])
            vh_sb = consts.tile([128, NROW, KK, NROW], bf16)
            nc.sync.dma_start(out=vh_sb, in_=vh[:, :, :, :])

            # Inline 0/1 masks for the softmax normalization matmuls:
            # bsum[b] = sum_n esums[8b+n]; brep[8b+n] = bsum[b].
            ma_np = np.zeros((64, BPC), np.float32)
            mb_np = np.zeros((BPC, 64), np.float32)
            for p in range(64):
                ma_np[p, p // NCH] = 1.0
                mb_np[p // NCH, p] = 1.0
            ma_dram = nc.inline_tensor(ma_np, name="ma")
            mb_dram = nc.inline_tensor(mb_np, name="mb")
            ma_sb = consts.tile([64, BPC], f32)
            nc.sync.dma_start(out=ma_sb, in_=ma_dram[:, :])
            mb_sb = consts.tile([BPC, 64], f32)
            nc.sync.dma_start(out=mb_sb, in_=mb_dram[:, :])

            # Per-batch bias: bias[m*128+h', b] = (W3 @ hidden[b])[m*128+h']
            bias_sb = consts.tile([128, KK, BPC], f32)
            for m in range(KK):
                bias_ps = spsum.tile([128, BPC], f32, tag="small")
                for kk in range(KK):
                    nc.tensor.matmul(
                        bias_ps,
                        lhsT=w3_sb[:, kk * 2 + m, :],
                        rhs=ht_sb[:, kk, :],
                        start=(kk == 0),
                        stop=(kk == KK - 1),
                    )
                nc.vector.tensor_copy(out=bias_sb[:, m, :], in_=bias_ps)

            loop_cm = (
                tc.For_i(0, loop_iters, 1) if loop_iters else contextlib.nullcontext()
            )
            with loop_cm:
              for _ in range(reps):
                # All 64 v-dot chunks accumulate into this bank; partition
                # p = 8*b + n comes from the one-hot lhsT column.
                scores_ps = vpsum.tile([NROW, 512], f32, tag="scores")
                pending = None

                def emit_vdot(pend):
                    # v-dot runs one chunk late so the tanh results are
                    # ready and the PE never waits on the ACT engine.
                    row, tts = pend
                    for m in range(KK):
                        nc.tensor.matmul(
                            scores_ps,
                            lhsT=vh_sb[:, row, m, :],
                            rhs=tts[m],
                            start=(row == 0 and m == 0),
                            stop=(row == NROW - 1 and m == KK - 1),
                        )

                for b in range(BPC):
                    # Stream the packed encoder quarters; each is one fully
                    # contiguous 512 KiB transfer (4 KiB per partition).
                    xt = []
                    for q in range(NQ):
                        xtile = xpool.tile([128, 2, 2, SQ], fp8, tag=f"x{q}")
                        nc.sync.dma_start(out=xtile, in_=xq[b, q])
                        xt.append(xtile)

                    for n in range(NCH):
                        q, r2 = divmod(n, NCH // NQ)
                        sl = slice(r2 * 512, (r2 + 1) * 512)
                        tts = []
                        for m in range(KK):
                            ps = mpsum.tile([128, 512], f32, tag="ps")
                            i = 0
                            for t in range(2):
                                for r in range(2):
                                    nc.tensor.matmul(
                                        ps,
                                        lhsT=wq_sb[:, t, r, :, m, :],
                                        rhs=xt[q][:, t, :, sl],
                                        start=(i == 0),
                                        stop=(i == 3),
                                        perf_mode=DR,
                                    )
                                    i += 1
                            tt = tpool.tile([128, 512], bf16, tag="tt")
                            nc.scalar.activation(
                                out=tt,
                                in_=ps,
                                func=mybir.ActivationFunctionType.Tanh,
                                bias=bias_sb[:, m, b : b + 1],
                                scale=INV_SCALE,
                            )
                            tts.append(tt)
                        if pending is not None:
                            emit_vdot(pending)
                        pending = (b * NCH + n, tts)
                # flush the last chunk's v-dot after the loop
                emit_vdot(pending)

                # Softmax epilogue. Scores are small (|s| < ~6), so skip the
                # max subtraction: softmax = exp(s) / sum(exp(s)). The
                # per-batch sums are formed from the per-partition accum via
                # two tiny 0/1-mask matmuls (sum over n, then broadcast).
                scores64 = spool.tile([64, 512], f32, tag="scores")
                esums = spool.tile([64, 1], f32, tag="esums")
                nc.scalar.activation(
                    out=scores64,
                    in_=scores_ps,
                    func=mybir.ActivationFunctionType.Exp,
                    accum_out=esums,
                )
                bsum_ps = spsum.tile([BPC, 1], f32, tag="small")
                nc.tensor.matmul(bsum_ps, lhsT=ma_sb, rhs=esums,
                                 start=True, stop=True)
                bsum_sb = spool.tile([BPC, 1], f32, tag="bsum")
                nc.vector.tensor_copy(out=bsum_sb, in_=bsum_ps)
                brep_ps = spsum.tile([64, 1], f32, tag="small")
                nc.tensor.matmul(brep_ps, lhsT=mb_sb, rhs=bsum_sb,
                                 start=True, stop=True)
                recip = spool.tile([64, 1], f32, tag="recip")
                nc.vector.reciprocal(out=recip, in_=brep_ps)
                nc.vector.tensor_scalar_mul(out=scores64, in0=scores64, scalar1=recip)
                nc.sync.dma_start(
                    out=out[:, :].rearrange("b (n s) -> (b n) s", n=NCH),
                    in_=scores64,
                )

    nc.finalize()
    return nc


def prep_shared_inputs(W: np.ndarray, v: np.ndarray, decoder_hidden: np.ndarray):
    """Host-side layout marshaling of the small replicated parameters."""
    W = np.ascontiguousarray(W, dtype=np.float32)

    # fp8 weight tiles + residuals for the two encoder blocks.
    wq_tiles = np.empty((128, 2, 2, 2, 2, 128), FP8)
    for t in range(2):
        Ws = W[:, t * H : (t + 1) * H] * W_SCALE        # [h, k] scaled
        Wq = Ws.astype(FP8)
        Wr = (Ws - Wq.astype(np.float32)).astype(FP8)
        for r, Wx in ((0, Wq), (1, Wr)):
            WxT = Wx.T                                   # [k, h]
            for i in range(KK):
                for m in range(KK):
                    wq_tiles[:, t, r, i, m, :] = WxT[
                        i * 128 : (i + 1) * 128, m * 128 : (m + 1) * 128
                    ]

    # W3 tiles in f32 for the bias path.
    w3_tiles = np.empty((128, 4, 128), np.float32)
    W3T = W[:, 2 * H :].T  # [k, h]
    for kk in range(KK):
        for m in range(KK):
            w3_tiles[:, kk * 2 + m, :] = W3T[
                kk * 128 : (kk + 1) * 128, m * 128 : (m + 1) * 128
            ]

    # One-hot v tiles: vh[p, row, m, c] = v[m*128+p] if c == row else 0.
    vt = v[0].reshape(KK, 128).T.astype(np.float32)      # [p, m]
    vh = np.zeros((128, NROW, KK, NROW), np.float32)
    idx = np.arange(NROW)
    vh[:, idx, :, idx] = vt[:, None, :].repeat(NROW, 1).transpose(1, 0, 2)
    vh = vh.astype(BF16)

    hT = decoder_hidden[0].T.astype(np.float32)  # [H, B]
    return wq_tiles, w3_tiles, vh, hT


def pack_encoder(xs: np.ndarray, xd: np.ndarray):
    """Quantize to fp8 (scaled by X_SCALE) and pack both encoder tensors into
    [B, q, p, t, i, sq] so each (b, q) slice is one contiguous transfer."""
    packed = np.empty((B, NQ, 128, 2, 2, SQ), FP8)
    for t, x in ((0, xs), (1, xd)):
        x8 = (x * np.float32(X_SCALE)).astype(FP8)        # [B, H, S]
        # [B, i, p, q, sq] -> [B, q, p, i, sq]
        v5 = x8.reshape(B, 2, 128, NQ, SQ).transpose(0, 3, 2, 1, 4)
        packed[:, :, :, t, :, :] = v5
    return packed


_CACHED = {}


def _get_nc(reps: int = 1, loop_iters: int = 0):
    key = (reps, loop_iters)
    if key not in _CACHED:
        _CACHED[key] = build_bass(reps, loop_iters)
    return _CACHED[key]


def make_in_maps(static_enc, dynamic_enc, decoder_hidden, W, v):
    wq_tiles, w3_tiles, vh, hT = prep_shared_inputs(W, v, decoder_hidden)
    static_enc = np.ascontiguousarray(static_enc, dtype=np.float32)
    dynamic_enc = np.ascontiguousarray(dynamic_enc, dtype=np.float32)
    xq_all = pack_encoder(static_enc, dynamic_enc)
    in_maps = []
    for c in range(N_CORES):
        b0 = c * BPC
        ht_c = np.ascontiguousarray(
            hT[:, b0 : b0 + BPC].reshape(KK, 128, BPC).transpose(1, 0, 2)
        )  # [p, kk, b]
        in_maps.append(
            {
                "xq": xq_all[b0 : b0 + BPC],
                "wq": wq_tiles,
                "w3": w3_tiles,
                "ht": ht_c,
                "vh": vh,
            }
        )
    return in_maps


def kernel(static_enc, dynamic_enc, decoder_hidden, W, v):
    from concourse.bass_utils import run_bass_kernel_spmd

    nc = _get_nc(reps=1)
    in_maps = make_in_maps(static_enc, dynamic_enc, decoder_hidden, W, v)
    res = run_bass_kernel_spmd(nc, in_maps, core_ids=list(range(N_CORES)))
    return np.concatenate([r["out"] for r in res.results], axis=0)


# revision 7
# speedup vs baseline: 1.2682x; 1.2682x over previous
"""Trainium2 Bass kernel for the sparse-attention scores module.

Computes, for each batch b:
    scores[b, :] = softmax_s( v . tanh(W1 @ static[b] + W2 @ dynamic[b] + W3 @ hidden[b]) )
with W = [W1 | W2 | W3] of shape [H, 3H], static/dynamic [B, H, S], hidden [B, H].

Sharding: data-parallel over B across 8 NeuronCores (8 batches per core).

The encoder streams are quantized to fp8 (e4m3) on the host, which halves the
HBM traffic vs bf16 and enables the PE's DoubleRow mode (0.5 cycles/row,
256-deep contraction per instruction). The [H,3H] weight is quantized to fp8
as a (Wq, Wr) pair — Wr is the fp8 quantization residual at the same
fixed-point scale — so accumulating both matmuls in PSUM removes the weight
quantization error entirely and only the encoder quantization error
(~1.4e-2 rel l2, measured) remains. Scales: x*8, W*128, undone by the ACT
engine's pre-tanh scale = 1/1024.

The v-dot runs in bf16 on the PE. Each (batch, chunk) v-dot uses a one-hot
weight tile (v in column 8*b+n, zeros elsewhere) so all 64 results accumulate
directly into one [64, 512] PSUM bank at the right partition — no per-chunk
staging copies or SBUF->SBUF DMAs. The softmax epilogue reads that bank.
"""

import sys

sys.path.insert(0, "/opt/trn_rl_repo")

import numpy as np
import ml_dtypes

B, H, S = 64, 256, 4096
N_CORES = 8
BPC = B // N_CORES          # batches per core
KK = H // 128               # 2 contraction chunks of the output dim
NCH = S // 512              # 8 psum column chunks
NQ = 4                      # input DMA quarters along s
SQ = S // NQ                # 1024 columns per quarter
NROW = BPC * NCH            # 64 score chunks = psum partitions

X_SCALE = 8.0
W_SCALE = 128.0
INV_SCALE = 1.0 / (X_SCALE * W_SCALE)

FP8 = ml_dtypes.float8_e4m3
BF16 = ml_dtypes.bfloat16


def build_bass(reps: int = 1, loop_iters: int = 0):
    """Build the per-core Bass program. reps>1 unrolls the whole computation
    multiple times; loop_iters>0 additionally wraps the unrolled body in a
    hardware loop. Both are used only for timing by differencing."""
    import contextlib

    import concourse.bacc as bacc
    import concourse.tile as tile
    from concourse import mybir

    f32 = mybir.dt.float32
    f32r = mybir.dt.float32r
    bf16 = mybir.dt.bfloat16
    fp8 = mybir.dt.float8e4
    DR = mybir.MatmulPerfMode.DoubleRow

    nc = bacc.Bacc(None)

    # Encoder data, fp8, packed as [b, q, p, t, i, sq]:
    #   element = x_t[b, h = i*128 + p, s = q*SQ + sq] * X_SCALE
    # so each (b, q) transfer is fully contiguous with 4 KiB per partition.
    xq = nc.dram_tensor("xq", [BPC, NQ, 128, 2, 2, SQ], fp8, kind="ExternalInput")
    # fp8 weights [p, t, r, i, m, mc]: lhsT tiles for DoubleRow;
    # r=0 is q8(W*W_SCALE), r=1 is the quantization residual (same scale).
    wq = nc.dram_tensor("wq", [128, 2, 2, 2, 2, 128], fp8, kind="ExternalInput")
    # W3 tiles (f32) for the per-batch bias, layout [p, kk*2+m, mc].
    w3 = nc.dram_tensor("w3", [128, 4, 128], f32r, kind="ExternalInput")
    ht = nc.dram_tensor("ht", [128, KK, BPC], f32r, kind="ExternalInput")
    # One-hot v tiles [p, row, m, c]: vh[p, row, m, c] = v[m*128+p] * (c==row).
    vh = nc.dram_tensor("vh", [128, NROW, KK, NROW], bf16, kind="ExternalInput")
    out = nc.dram_tensor("out", [BPC, S], f32, kind="ExternalOutput")

    with tile.TileContext(nc) as tc:
        with (
            tc.tile_pool(name="consts", bufs=1) as consts,
            tc.tile_pool(name="xpool", bufs=10) as xpool,
            tc.tile_pool(name="tpool", bufs=10) as tpool,
            tc.tile_pool(name="spool", bufs=2) as spool,
            tc.tile_pool(name="mpsum", bufs=4, space="PSUM") as mpsum,
            tc.tile_pool(name="vpsum", bufs=2, space="PSUM") as vpsum,
            tc.tile_pool(name="spsum", bufs=2, space="PSUM") as spsum,
        ):
            wq_sb = consts.tile([128, 2, 2, 2, 2, 128], fp8)
            nc.sync.dma_start(out=wq_sb, in_=wq[:, :, :, :, :, :])
            w3_sb = consts.tile([128, 4, 128], f32r)
            nc.sync.dma_start(out=w3_sb, in_=w3[:, :, :])
            ht_sb = consts.tile([128, KK, BPC], f32r)
            nc.sync.dma_start(out=ht_sb, in_=ht[:, :, :])
            vh_sb = consts.tile([128, NROW, KK, NROW], bf16)
            nc.sync.dma_start(out=vh_sb, in_=vh[:, :, :, :])

            # Inline 0/1 masks for the softmax normalization matmuls:
            # bsum[b] = sum_n esums[8b+n]; brep[8b+n] = bsum[b].
            ma_np = np.zeros((64, BPC), np.float32)
            mb_np = np.zeros((BPC, 64), np.float32)
            for p in range(64):
                ma_np[p, p // NCH] = 1.0
                mb_np[p // NCH, p] = 1.0
            ma_dram = nc.inline_tensor(ma_np, name="ma")
            mb_dram = nc.inline_tensor(mb_np, name="mb")
            ma_sb = consts.tile([64, BPC], f32)
            nc.sync.dma_start(out=ma_sb, in_=ma_dram[:, :])
            mb_sb = consts.tile([BPC, 64], f32)
            nc.sync.dma_start(out=mb_sb, in_=mb_dram[:, :])

            # Per-batch bias: bias[m*128+h', b] = (W3 @ hidden[b])[m*128+h']
            bias_sb = consts.tile([128, KK, BPC], f32)
            for m in range(KK):
                bias_ps = spsum.tile([128, BPC], f32, tag="small")
                for kk in range(KK):
                    nc.tensor.matmul(
                        bias_ps,
                        lhsT=w3_sb[:, kk * 2 + m, :],
                        rhs=ht_sb[:, kk, :],
                        start=(kk == 0),
                        stop=(kk == KK - 1),
                    )
                nc.vector.tensor_copy(out=bias_sb[:, m, :], in_=bias_ps)

            loop_cm = (
                tc.For_i(0, loop_iters, 1) if loop_iters else contextlib.nullcontext()
            )
            with loop_cm:
              for _ in range(reps):
                # All 64 v-dot chunks accumulate into this bank; partition
                # p = 8*b + n comes from the one-hot lhsT column.
                scores_ps = vpsum.tile([NROW, 512], f32, tag="scores")
                pending = []

                def emit_vdot(pend):
                    # v-dots run a full quarter late so the tanh results are
                    # long since ready: the PE never stalls at the queue head
                    # waiting on the ACT semaphores, and the one-hot weight
                    # loads get pulled ahead into the background buffer.
                    row, tts = pend
                    for m in range(KK):
                        nc.tensor.matmul(
                            scores_ps,
                            lhsT=vh_sb[:, row, m, :],
                            rhs=tts[m],
                            start=(row == 0 and m == 0),
                            stop=(row == NROW - 1 and m == KK - 1),
                        )

                for b in range(BPC):
                    # Stream the packed encoder quarters; each is one fully
                    # contiguous 512 KiB transfer (4 KiB per partition).
                    xt = []
                    for q in range(NQ):
                        xtile = xpool.tile([128, 2, 2, SQ], fp8, tag="x")
                        nc.sync.dma_start(out=xtile, in_=xq[b, q])
                        xt.append(xtile)

                    for q in range(NQ):
                        # Weight-major order: each DoubleRow weight load (256
                        # columns, ~213 ns — the expensive part) is amortized
                        # over both 512-column chunks of the quarter, so the
                        # PE pipeline stays balanced (~213 ns load covering
                        # ~214 ns of matmul).
                        ps = [
                            [
                                mpsum.tile([128, 512], f32, tag="ps", name=f"ps_{m}_{c}")
                                for c in range(2)
                            ]
                            for m in range(KK)
                        ]
                        tts_q = []
                        for m in range(KK):
                            i = 0
                            for t in range(2):
                                for r in range(2):
                                    for c in range(2):
                                        nc.tensor.matmul(
                                            ps[m][c],
                                            lhsT=wq_sb[:, t, r, :, m, :],
                                            rhs=xt[q][:, t, :, c * 512 : (c + 1) * 512],
                                            start=(i == 0),
                                            stop=(i == 3),
                                            perf_mode=DR,
                                            skip_group_check=True,
                                        )
                                    i += 1
                            for c in range(2):
                                tt = tpool.tile([128, 512], bf16, tag="tt")
                                nc.scalar.activation(
                                    out=tt,
                                    in_=ps[m][c],
                                    func=mybir.ActivationFunctionType.Tanh,
                                    bias=bias_sb[:, m, b : b + 1],
                                    scale=INV_SCALE,
                                )
                                tts_q.append((m, c, tt))
                        for c in range(2):
                            row = b * NCH + q * 2 + c
                            tts = [tt for (m, cc, tt) in tts_q if cc == c]
                            pending.append((row, tts))
                        while len(pending) > 2:
                            emit_vdot(pending.pop(0))
                # flush the remaining v-dots after the loop
                for pend in pending:
                    emit_vdot(pend)

                # Softmax epilogue. Scores are small (|s| < ~6), so skip the
                # max subtraction: softmax = exp(s) / sum(exp(s)). The
                # per-batch sums are formed from the per-partition accum via
                # two tiny 0/1-mask matmuls (sum over n, then broadcast).
                scores64 = spool.tile([64, 512], f32, tag="scores")
                esums = spool.tile([64, 1], f32, tag="esums")
                nc.scalar.activation(
                    out=scores64,
                    in_=scores_ps,
                    func=mybir.ActivationFunctionType.Exp,
                    accum_out=esums,
                )
                bsum_ps = spsum.tile([BPC, 1], f32, tag="small")
                nc.tensor.matmul(bsum_ps, lhsT=ma_sb, rhs=esums,
                                 start=True, stop=True)
                bsum_sb = spool.tile([BPC, 1], f32, tag="bsum")
                nc.vector.tensor_copy(out=bsum_sb, in_=bsum_ps)
                brep_ps = spsum.tile([64, 1], f32, tag="small")
                nc.tensor.matmul(brep_ps, lhsT=mb_sb, rhs=bsum_sb,
                                 start=True, stop=True)
                recip = spool.tile([64, 1], f32, tag="recip")
                nc.vector.reciprocal(out=recip, in_=brep_ps)
                nc.vector.tensor_scalar_mul(out=scores64, in0=scores64, scalar1=recip)
                nc.sync.dma_start(
                    out=out[:, :].rearrange("b (n s) -> (b n) s", n=NCH),
                    in_=scores64,
                )

    nc.finalize()
    return nc


def prep_shared_inputs(W: np.ndarray, v: np.ndarray, decoder_hidden: np.ndarray):
    """Host-side layout marshaling of the small replicated parameters."""
    W = np.ascontiguousarray(W, dtype=np.float32)

    # fp8 weight tiles + residuals for the two encoder blocks.
    wq_tiles = np.empty((128, 2, 2, 2, 2, 128), FP8)
    for t in range(2):
        Ws = W[:, t * H : (t + 1) * H] * W_SCALE        # [h, k] scaled
        Wq = Ws.astype(FP8)
        Wr = (Ws - Wq.astype(np.float32)).astype(FP8)
        for r, Wx in ((0, Wq), (1, Wr)):
            WxT = Wx.T                                   # [k, h]
            for i in range(KK):
                for m in range(KK):
                    wq_tiles[:, t, r, i, m, :] = WxT[
                        i * 128 : (i + 1) * 128, m * 128 : (m + 1) * 128
                    ]

    # W3 tiles in f32 for the bias path.
    w3_tiles = np.empty((128, 4, 128), np.float32)
    W3T = W[:, 2 * H :].T  # [k, h]
    for kk in range(KK):
        for m in range(KK):
            w3_tiles[:, kk * 2 + m, :] = W3T[
                kk * 128 : (kk + 1) * 128, m * 128 : (m + 1) * 128
            ]

    # One-hot v tiles: vh[p, row, m, c] = v[m*128+p] if c == row else 0.
    vt = v[0].reshape(KK, 128).T.astype(np.float32)      # [p, m]
    vh = np.zeros((128, NROW, KK, NROW), np.float32)
    idx = np.arange(NROW)
    vh[:, idx, :, idx] = vt[:, None, :].repeat(NROW, 1).transpose(1, 0, 2)
    vh = vh.astype(BF16)

    hT = decoder_hidden[0].T.astype(np.float32)  # [H, B]
    return wq_tiles, w3_tiles, vh, hT


def pack_encoder(xs: np.ndarray, xd: np.ndarray):
    """Quantize to fp8 (scaled by X_SCALE) and pack both encoder tensors into
    [B, q, p, t, i, sq] so each (b, q) slice is one contiguous transfer."""
    packed = np.empty((B, NQ, 128, 2, 2, SQ), FP8)
    for t, x in ((0, xs), (1, xd)):
        x8 = (x * np.float32(X_SCALE)).astype(FP8)        # [B, H, S]
        # [B, i, p, q, sq] -> [B, q, p, i, sq]
        v5 = x8.reshape(B, 2, 128, NQ, SQ).transpose(0, 3, 2, 1, 4)
        packed[:, :, :, t, :, :] = v5
    return packed


_CACHED = {}


def _get_nc(reps: int = 1, loop_iters: int = 0):
    key = (reps, loop_iters)
    if key not in _CACHED:
        _CACHED[key] = build_bass(reps, loop_iters)
    return _CACHED[key]


def make_in_maps(static_enc, dynamic_enc, decoder_hidden, W, v):
    wq_tiles, w3_tiles, vh, hT = prep_shared_inputs(W, v, decoder_hidden)
    static_enc = np.ascontiguousarray(static_enc, dtype=np.float32)
    dynamic_enc = np.ascontiguousarray(dynamic_enc, dtype=np.float32)
    xq_all = pack_encoder(static_enc, dynamic_enc)
    in_maps = []
    for c in range(N_CORES):
        b0 = c * BPC
        ht_c = np.ascontiguousarray(
            hT[:, b0 : b0 + BPC].reshape(KK, 128, BPC).transpose(1, 0, 2)
        )  # [p, kk, b]
        in_maps.append(
            {
                "xq": xq_all[b0 : b0 + BPC],
                "wq": wq_tiles,
                "w3": w3_tiles,
                "ht": ht_c,
                "vh": vh,
            }
        )
    return in_maps


def kernel(static_enc, dynamic_enc, decoder_hidden, W, v):
    from concourse.bass_utils import run_bass_kernel_spmd

    nc = _get_nc(reps=1)
    in_maps = make_in_maps(static_enc, dynamic_enc, decoder_hidden, W, v)
    res = run_bass_kernel_spmd(nc, in_maps, core_ids=list(range(N_CORES)))
    return np.concatenate([r["out"] for r in res.results], axis=0)


# revision 14
# speedup vs baseline: 1.3292x; 1.0481x over previous
"""Trainium2 Bass kernel for the sparse-attention scores module.

Computes, for each batch b:
    scores[b, :] = softmax_s( v . tanh(W1 @ static[b] + W2 @ dynamic[b] + W3 @ hidden[b]) )
with W = [W1 | W2 | W3] of shape [H, 3H], static/dynamic [B, H, S], hidden [B, H].

Sharding: data-parallel over B across 8 NeuronCores (8 batches per core).

The encoder streams are quantized to fp8 (e4m3) on the host, which halves the
HBM traffic vs bf16 and enables the PE's DoubleRow mode (256-deep contraction
per instruction). On TRN2 each DoubleRow matmul is bound by its 256-column
weight load (~213 ns), so instruction count is what matters: W1 is quantized
as a (Wq, Wr) pair — Wr being the fp8 quantization residual at the same
fixed-point scale, accumulated in the same PSUM group — while W2 keeps only
Wq; that is 3 DoubleRow matmuls per output tile. Measured end-to-end error
1.76e-2 rel l2 (gate 2e-2); flip RESIDUAL_BOTH for 1.38e-2 at +28 us.
Scales: x*8, W*128, undone by the ACT engine's pre-tanh scale = 1/1024.

The v-dot runs in bf16 on the PE into a [1,512] PSUM chunk (one-column
weights load in ~1 ns); the DVE drains chunks into a per-batch [1,4096] row
and one SBUF->SBUF DMA per batch places it across the 8 score partitions of
that batch, so the softmax epilogue runs on all 64 partitions at once.
"""

import sys

sys.path.insert(0, "/opt/trn_rl_repo")

import numpy as np
import ml_dtypes

B, H, S = 64, 256, 4096
N_CORES = 8
BPC = B // N_CORES          # batches per core
KK = H // 128               # 2 contraction chunks of the output dim
NCH = S // 512              # 8 psum column chunks
NQ = 4                      # input DMA quarters along s
SQ = S // NQ                # 1024 columns per quarter

X_SCALE = 8.0
W_SCALE = 128.0
INV_SCALE = 1.0 / (X_SCALE * W_SCALE)
RESIDUAL_BOTH = False       # True: residual for W2 as well (err 1.38e-2)

FP8 = ml_dtypes.float8_e4m3
BF16 = ml_dtypes.bfloat16


def build_bass(reps: int = 1, loop_iters: int = 0):
    """Build the per-core Bass program. reps>1 unrolls the whole computation
    multiple times; loop_iters>0 additionally wraps the unrolled body in a
    hardware loop. Both are used only for timing by differencing."""
    import contextlib

    import concourse.bacc as bacc
    import concourse.tile as tile
    from concourse import mybir

    f32 = mybir.dt.float32
    f32r = mybir.dt.float32r
    bf16 = mybir.dt.bfloat16
    fp8 = mybir.dt.float8e4
    DR = mybir.MatmulPerfMode.DoubleRow

    # (tensor, use-residual) pairs for the main matmul weight sweep.
    wsel = [(0, 0), (0, 1), (1, 0)] + ([(1, 1)] if RESIDUAL_BOTH else [])

    nc = bacc.Bacc(None)

    # Encoder data, fp8, packed as [b, q, p, t, i, sq]:
    #   element = x_t[b, h = i*128 + p, s = q*SQ + sq] * X_SCALE
    # so each (b, q) transfer is fully contiguous with 4 KiB per partition.
    xq = nc.dram_tensor("xq", [BPC, NQ, 128, 2, 2, SQ], fp8, kind="ExternalInput")
    # fp8 weights [p, t, r, i, m, mc]: lhsT tiles for DoubleRow;
    # r=0 is q8(W*W_SCALE), r=1 is the quantization residual (same scale).
    wq = nc.dram_tensor("wq", [128, 2, 2, 2, 2, 128], fp8, kind="ExternalInput")
    # W3 tiles (f32) for the per-batch bias, layout [p, kk*2+m, mc].
    w3 = nc.dram_tensor("w3", [128, 4, 128], f32r, kind="ExternalInput")
    ht = nc.dram_tensor("ht", [128, KK, BPC], f32r, kind="ExternalInput")
    vt = nc.dram_tensor("vt", [128, KK], bf16, kind="ExternalInput")
    out = nc.dram_tensor("out", [BPC, S], f32, kind="ExternalOutput")

    with tile.TileContext(nc) as tc:
        with (
            tc.tile_pool(name="consts", bufs=1) as consts,
            tc.tile_pool(name="xpool", bufs=10) as xpool,
            tc.tile_pool(name="tpool", bufs=10) as tpool,
            tc.tile_pool(name="spool", bufs=2) as spool,
            tc.tile_pool(name="mpsum", bufs=4, space="PSUM") as mpsum,
            tc.tile_pool(name="vpsum", bufs=2, space="PSUM") as vpsum,
            tc.tile_pool(name="spsum", bufs=2, space="PSUM") as spsum,
        ):
            wq_sb = consts.tile([128, 2, 2, 2, 2, 128], fp8)
            nc.sync.dma_start(out=wq_sb, in_=wq[:, :, :, :, :, :])
            w3_sb = consts.tile([128, 4, 128], f32r)
            nc.sync.dma_start(out=w3_sb, in_=w3[:, :, :])
            ht_sb = consts.tile([128, KK, BPC], f32r)
            nc.sync.dma_start(out=ht_sb, in_=ht[:, :, :])
            vt_sb = consts.tile([128, KK], bf16)
            nc.sync.dma_start(out=vt_sb, in_=vt[:, :])

            # Inline 0/1 masks for the softmax normalization matmuls:
            # bsum[b] = sum_n esums[8b+n]; brep[8b+n] = bsum[b].
            ma_np = np.zeros((64, BPC), np.float32)
            mb_np = np.zeros((BPC, 64), np.float32)
            for p in range(64):
                ma_np[p, p // NCH] = 1.0
                mb_np[p // NCH, p] = 1.0
            ma_dram = nc.inline_tensor(ma_np, name="ma")
            mb_dram = nc.inline_tensor(mb_np, name="mb")
            ma_sb = consts.tile([64, BPC], f32)
            nc.sync.dma_start(out=ma_sb, in_=ma_dram[:, :])
            mb_sb = consts.tile([BPC, 64], f32)
            nc.sync.dma_start(out=mb_sb, in_=mb_dram[:, :])

            # Per-batch bias: bias[m*128+h', b] = (W3 @ hidden[b])[m*128+h']
            bias_sb = consts.tile([128, KK, BPC], f32)
            for m in range(KK):
                bias_ps = spsum.tile([128, BPC], f32, tag="small")
                for kk in range(KK):
                    nc.tensor.matmul(
                        bias_ps,
                        lhsT=w3_sb[:, kk * 2 + m, :],
                        rhs=ht_sb[:, kk, :],
                        start=(kk == 0),
                        stop=(kk == KK - 1),
                    )
                nc.vector.tensor_copy(out=bias_sb[:, m, :], in_=bias_ps)

            loop_cm = (
                tc.For_i(0, loop_iters, 1) if loop_iters else contextlib.nullcontext()
            )
            with loop_cm:
              for _ in range(reps):
                # Scores live as [64, 512] with partition p = 8*b + n so the
                # epilogue runs on all 64 partitions at once.
                scores64 = spool.tile([64, 512], f32, tag="scores")
                pending = None

                def emit_vdot(pend):
                    # v-dot runs one chunk late so the tanh results are
                    # ready and the PE never waits on the ACT engine.
                    b, n, tts = pend
                    row = b * NCH + n
                    vp = vpsum.tile([1, 512], f32, tag="vp")
                    for m in range(KK):
                        nc.tensor.matmul(
                            vp,
                            lhsT=vt_sb[:, m : m + 1],
                            rhs=tts[m],
                            start=(m == 0),
                            stop=(m == KK - 1),
                        )
                    # Compute engines may only address partition bases that
                    # are multiples of 32, so the chunk is drained to
                    # partition 0 and a tiny SBUF->SBUF DMA places it at
                    # partition 8b+n of the scores tile. The DMAs alternate
                    # between the Pool queue (~1.5 us Q7 launch each) and the
                    # SP queue so neither becomes the bottleneck.
                    stage = tpool.tile([1, 512], f32, tag="stage")
                    nc.vector.tensor_copy(out=stage, in_=vp)
                    queue = nc.gpsimd if row % 2 == 0 else nc.sync
                    queue.dma_start(
                        out=scores64[row : row + 1, :],
                        in_=stage,
                    )

                for b in range(BPC):
                    # Stream the packed encoder quarters; each is one fully
                    # contiguous 512 KiB transfer (4 KiB per partition).
                    xt = []
                    for q in range(NQ):
                        xtile = xpool.tile([128, 2, 2, SQ], fp8, tag="x")
                        nc.sync.dma_start(out=xtile, in_=xq[b, q])
                        xt.append(xtile)

                    for q in range(NQ):
                        # Weight-major order: each DoubleRow weight load (the
                        # expensive part, ~213 ns for 256 columns) covers both
                        # 512-column chunks of the quarter.
                        ps = [
                            [
                                mpsum.tile([128, 512], f32, tag="ps", name=f"ps_{m}_{c}")
                                for c in range(2)
                            ]
                            for m in range(KK)
                        ]
                        tts_q = []
                        for m in range(KK):
                            for i, (t, r) in enumerate(wsel):
                                for c in range(2):
                                    nc.tensor.matmul(
                                        ps[m][c],
                                        lhsT=wq_sb[:, t, r, :, m, :],
                                        rhs=xt[q][:, t, :, c * 512 : (c + 1) * 512],
                                        start=(i == 0),
                                        stop=(i == len(wsel) - 1),
                                        perf_mode=DR,
                                        skip_group_check=True,
                                    )
                            for c in range(2):
                                tt = tpool.tile([128, 512], bf16, tag="tt")
                                nc.scalar.activation(
                                    out=tt,
                                    in_=ps[m][c],
                                    func=mybir.ActivationFunctionType.Tanh,
                                    bias=bias_sb[:, m, b : b + 1],
                                    scale=INV_SCALE,
                                )
                                tts_q.append((m, c, tt))
                        for c in range(2):
                            if pending is not None:
                                emit_vdot(pending)
                            n = q * 2 + c
                            tts = [tt for (m, cc, tt) in tts_q if cc == c]
                            pending = (b, n, tts)
                # flush the last chunk's v-dot after the loop
                emit_vdot(pending)
                pending = None

                # Softmax epilogue. Scores are small (|s| < ~6), so skip the
                # max subtraction: softmax = exp(s) / sum(exp(s)). The
                # per-batch sums are formed from the per-partition accum via
                # two tiny 0/1-mask matmuls (sum over n, then broadcast).
                esums = spool.tile([64, 1], f32, tag="esums")
                nc.scalar.activation(
                    out=scores64,
                    in_=scores64,
                    func=mybir.ActivationFunctionType.Exp,
                    accum_out=esums,
                )
                bsum_ps = spsum.tile([BPC, 1], f32, tag="small")
                nc.tensor.matmul(bsum_ps, lhsT=ma_sb, rhs=esums,
                                 start=True, stop=True)
                bsum_sb = spool.tile([BPC, 1], f32, tag="bsum")
                nc.vector.tensor_copy(out=bsum_sb, in_=bsum_ps)
                brep_ps = spsum.tile([64, 1], f32, tag="small")
                nc.tensor.matmul(brep_ps, lhsT=mb_sb, rhs=bsum_sb,
                                 start=True, stop=True)
                recip = spool.tile([64, 1], f32, tag="recip")
                nc.vector.reciprocal(out=recip, in_=brep_ps)
                nc.vector.tensor_scalar_mul(out=scores64, in0=scores64, scalar1=recip)
                nc.gpsimd.dma_start(
                    out=out[:, :].rearrange("b (n s) -> (b n) s", n=NCH),
                    in_=scores64,
                )

    nc.finalize()
    return nc


def prep_shared_inputs(W: np.ndarray, v: np.ndarray, decoder_hidden: np.ndarray):
    """Host-side layout marshaling of the small replicated parameters."""
    W = np.ascontiguousarray(W, dtype=np.float32)

    # fp8 weight tiles + residuals for the two encoder blocks.
    wq_tiles = np.empty((128, 2, 2, 2, 2, 128), FP8)
    for t in range(2):
        Ws = W[:, t * H : (t + 1) * H] * W_SCALE        # [h, k] scaled
        Wq = Ws.astype(FP8)
        Wr = (Ws - Wq.astype(np.float32)).astype(FP8)
        for r, Wx in ((0, Wq), (1, Wr)):
            WxT = Wx.T                                   # [k, h]
            for i in range(KK):
                for m in range(KK):
                    wq_tiles[:, t, r, i, m, :] = WxT[
                        i * 128 : (i + 1) * 128, m * 128 : (m + 1) * 128
                    ]

    # W3 tiles in f32 for the bias path.
    w3_tiles = np.empty((128, 4, 128), np.float32)
    W3T = W[:, 2 * H :].T  # [k, h]
    for kk in range(KK):
        for m in range(KK):
            w3_tiles[:, kk * 2 + m, :] = W3T[
                kk * 128 : (kk + 1) * 128, m * 128 : (m + 1) * 128
            ]

    vt = np.ascontiguousarray(v[0].reshape(KK, 128).T.astype(BF16))  # [p, kk]
    hT = decoder_hidden[0].T.astype(np.float32)  # [H, B]
    return wq_tiles, w3_tiles, vt, hT


def pack_encoder(xs: np.ndarray, xd: np.ndarray):
    """Quantize to fp8 (scaled by X_SCALE) and pack both encoder tensors into
    [B, q, p, t, i, sq] so each (b, q) slice is one contiguous transfer."""
    packed = np.empty((B, NQ, 128, 2, 2, SQ), FP8)
    for t, x in ((0, xs), (1, xd)):
        x8 = (x * np.float32(X_SCALE)).astype(FP8)        # [B, H, S]
        # [B, i, p, q, sq] -> [B, q, p, i, sq]
        v5 = x8.reshape(B, 2, 128, NQ, SQ).transpose(0, 3, 2, 1, 4)
        packed[:, :, :, t, :, :] = v5
    return packed


_CACHED = {}


def _get_nc(reps: int = 1, loop_iters: int = 0):
    key = (reps, loop_iters)
    if key not in _CACHED:
        _CACHED[key] = build_bass(reps, loop_iters)
    return _CACHED[key]


def make_in_maps(static_enc, dynamic_enc, decoder_hidden, W, v):
    wq_tiles, w3_tiles, vt, hT = prep_shared_inputs(W, v, decoder_hidden)
    static_enc = np.ascontiguousarray(static_enc, dtype=np.float32)
    dynamic_enc = np.ascontiguousarray(dynamic_enc, dtype=np.float32)
    xq_all = pack_encoder(static_enc, dynamic_enc)
    in_maps = []
    for c in range(N_CORES):
        b0 = c * BPC
        ht_c = np.ascontiguousarray(
            hT[:, b0 : b0 + BPC].reshape(KK, 128, BPC).transpose(1, 0, 2)
        )  # [p, kk, b]
        in_maps.append(
            {
                "xq": xq_all[b0 : b0 + BPC],
                "wq": wq_tiles,
                "w3": w3_tiles,
                "ht": ht_c,
                "vt": vt,
            }
        )
    return in_maps


def kernel(static_enc, dynamic_enc, decoder_hidden, W, v):
    from concourse.bass_utils import run_bass_kernel_spmd

    nc = _get_nc(reps=1)
    in_maps = make_in_maps(static_enc, dynamic_enc, decoder_hidden, W, v)
    res = run_bass_kernel_spmd(nc, in_maps, core_ids=list(range(N_CORES)))
    return np.concatenate([r["out"] for r in res.results], axis=0)


# revision 18
# speedup vs baseline: 1.3700x; 1.0307x over previous
"""Trainium2 Bass kernel for the sparse-attention scores module.

Computes, for each batch b:
    scores[b, :] = softmax_s( v . tanh(W1 @ static[b] + W2 @ dynamic[b] + W3 @ hidden[b]) )
with W = [W1 | W2 | W3] of shape [H, 3H], static/dynamic [B, H, S], hidden [B, H].

Sharding: data-parallel over B across 8 NeuronCores (8 batches per core).

The encoder streams are quantized to fp8 (e4m3) on the host, which halves the
HBM traffic vs bf16 and enables the PE's DoubleRow mode (256-deep contraction
per instruction). On TRN2 each DoubleRow matmul is bound by its 256-column
weight load (~213 ns), so instruction count is what matters: W1 is quantized
as a (Wq, Wr) pair — Wr being the fp8 quantization residual at the same
fixed-point scale, accumulated in the same PSUM group — while W2 keeps only
Wq; that is 3 DoubleRow matmuls per output tile. Measured end-to-end error
1.76e-2 rel l2 (gate 2e-2); flip RESIDUAL_BOTH for 1.38e-2 at +28 us.
Scales: x*8, W*128, undone by the ACT engine's pre-tanh scale = 1/1024.

The v-dot runs in bf16 on the PE into a [1,512] PSUM chunk (one-column
weights load in ~1 ns); the DVE drains chunks into a per-batch [1,4096] row
and one SBUF->SBUF DMA per batch places it across the 8 score partitions of
that batch, so the softmax epilogue runs on all 64 partitions at once.
"""

import sys

sys.path.insert(0, "/opt/trn_rl_repo")

import numpy as np
import ml_dtypes

B, H, S = 64, 256, 4096
N_CORES = 8
BPC = B // N_CORES          # batches per core
KK = H // 128               # 2 contraction chunks of the output dim
NCH = S // 512              # 8 psum column chunks
NQ = 4                      # input DMA quarters along s
SQ = S // NQ                # 1024 columns per quarter

X_SCALE = 8.0
W_SCALE = 128.0
INV_SCALE = 1.0 / (X_SCALE * W_SCALE)
RESIDUAL_BOTH = False       # True: residual for W2 as well (err 1.38e-2)

FP8 = ml_dtypes.float8_e4m3
BF16 = ml_dtypes.bfloat16


def build_bass(reps: int = 1, loop_iters: int = 0):
    """Build the per-core Bass program. reps>1 unrolls the whole computation
    multiple times; loop_iters>0 additionally wraps the unrolled body in a
    hardware loop. Both are used only for timing by differencing."""
    import contextlib

    import concourse.bacc as bacc
    import concourse.tile as tile
    from concourse import mybir

    f32 = mybir.dt.float32
    f32r = mybir.dt.float32r
    bf16 = mybir.dt.bfloat16
    fp8 = mybir.dt.float8e4
    DR = mybir.MatmulPerfMode.DoubleRow

    # (tensor, use-residual) pairs for the main matmul weight sweep.
    wsel = [(0, 0), (0, 1), (1, 0)] + ([(1, 1)] if RESIDUAL_BOTH else [])

    nc = bacc.Bacc(None)

    # Encoder data, fp8, packed as [b, q, p, t, i, sq]:
    #   element = x_t[b, h = i*128 + p, s = q*SQ + sq] * X_SCALE
    # so each (b, q) transfer is fully contiguous with 4 KiB per partition.
    xq = nc.dram_tensor("xq", [BPC, NQ, 128, 2, 2, SQ], fp8, kind="ExternalInput")
    # fp8 weights [p, t, r, i, m, mc]: lhsT tiles for DoubleRow;
    # r=0 is q8(W*W_SCALE), r=1 is the quantization residual (same scale).
    wq = nc.dram_tensor("wq", [128, 2, 2, 2, 2, 128], fp8, kind="ExternalInput")
    # W3 tiles (f32) for the per-batch bias, layout [p, kk*2+m, mc].
    w3 = nc.dram_tensor("w3", [128, 4, 128], f32r, kind="ExternalInput")
    ht = nc.dram_tensor("ht", [128, KK, BPC], f32r, kind="ExternalInput")
    vt = nc.dram_tensor("vt", [128, KK], bf16, kind="ExternalInput")
    out = nc.dram_tensor("out", [BPC, S], f32, kind="ExternalOutput")

    with tile.TileContext(nc) as tc:
        with (
            tc.tile_pool(name="consts", bufs=1) as consts,
            tc.tile_pool(name="xpool", bufs=10) as xpool,
            tc.tile_pool(name="tpool", bufs=10) as tpool,
            tc.tile_pool(name="spool", bufs=2) as spool,
            tc.tile_pool(name="mpsum", bufs=4, space="PSUM") as mpsum,
            tc.tile_pool(name="vpsum", bufs=2, space="PSUM") as vpsum,
            tc.tile_pool(name="spsum", bufs=2, space="PSUM") as spsum,
        ):
            wq_sb = consts.tile([128, 2, 2, 2, 2, 128], fp8)
            nc.sync.dma_start(out=wq_sb, in_=wq[:, :, :, :, :, :])
            w3_sb = consts.tile([128, 4, 128], f32r)
            nc.sync.dma_start(out=w3_sb, in_=w3[:, :, :])
            ht_sb = consts.tile([128, KK, BPC], f32r)
            nc.sync.dma_start(out=ht_sb, in_=ht[:, :, :])
            vt_sb = consts.tile([128, KK], bf16)
            nc.sync.dma_start(out=vt_sb, in_=vt[:, :])

            # Inline 0/1 masks for the softmax normalization matmuls:
            # bsum[b] = sum_n esums[8b+n]; brep[8b+n] = bsum[b].
            ma_np = np.zeros((64, BPC), np.float32)
            mb_np = np.zeros((BPC, 64), np.float32)
            for p in range(64):
                ma_np[p, p // NCH] = 1.0
                mb_np[p // NCH, p] = 1.0
            ma_dram = nc.inline_tensor(ma_np, name="ma")
            mb_dram = nc.inline_tensor(mb_np, name="mb")
            ma_sb = consts.tile([64, BPC], f32)
            nc.sync.dma_start(out=ma_sb, in_=ma_dram[:, :])
            mb_sb = consts.tile([BPC, 64], f32)
            nc.sync.dma_start(out=mb_sb, in_=mb_dram[:, :])

            # Per-batch bias: bias[m*128+h', b] = (W3 @ hidden[b])[m*128+h']
            bias_sb = consts.tile([128, KK, BPC], f32)
            for m in range(KK):
                bias_ps = spsum.tile([128, BPC], f32, tag="small")
                for kk in range(KK):
                    nc.tensor.matmul(
                        bias_ps,
                        lhsT=w3_sb[:, kk * 2 + m, :],
                        rhs=ht_sb[:, kk, :],
                        start=(kk == 0),
                        stop=(kk == KK - 1),
                    )
                nc.vector.tensor_copy(out=bias_sb[:, m, :], in_=bias_ps)

            loop_cm = (
                tc.For_i(0, loop_iters, 1) if loop_iters else contextlib.nullcontext()
            )
            with loop_cm:
              for _ in range(reps):
                # Scores live as [64, 512] with partition p = 8*b + n so the
                # epilogue runs on all 64 partitions at once.
                scores64 = spool.tile([64, 512], f32, tag="scores")
                pending = []

                def emit_vdot(pend):
                    # v-dots run two chunks late so the tanh results are
                    # long since ready and the PE never waits on the ACT
                    # engine.
                    b, n, tts = pend
                    row = b * NCH + n
                    vp = vpsum.tile([1, 512], f32, tag="vp")
                    for m in range(KK):
                        nc.tensor.matmul(
                            vp,
                            lhsT=vt_sb[:, m : m + 1],
                            rhs=tts[m],
                            start=(m == 0),
                            stop=(m == KK - 1),
                        )
                    # Compute engines may only address partition bases that
                    # are multiples of 32, so the chunk is drained to
                    # partition 0 and a tiny SBUF->SBUF DMA places it at
                    # partition 8b+n of the scores tile. The DMAs alternate
                    # between the Pool queue (~1.5 us Q7 launch each) and
                    # the SP queue so neither becomes the bottleneck.
                    stage = tpool.tile([1, 512], f32, tag="stage")
                    nc.vector.tensor_copy(out=stage, in_=vp)
                    queue = nc.gpsimd if row % 2 == 0 else nc.sync
                    queue.dma_start(
                        out=scores64[row : row + 1, :],
                        in_=stage,
                    )

                for b in range(BPC):
                    # Stream the packed encoder quarters; each is one fully
                    # contiguous 512 KiB transfer (4 KiB per partition).
                    xt = []
                    for q in range(NQ):
                        xtile = xpool.tile([128, 2, 2, SQ], fp8, tag="x")
                        nc.sync.dma_start(out=xtile, in_=xq[b, q])
                        xt.append(xtile)

                    for q in range(NQ):
                        # Weight-major order: each DoubleRow weight load (the
                        # expensive part, ~213 ns for 256 columns) covers both
                        # 512-column chunks of the quarter.
                        ps = [
                            [
                                mpsum.tile([128, 512], f32, tag="ps", name=f"ps_{m}_{c}")
                                for c in range(2)
                            ]
                            for m in range(KK)
                        ]
                        tts_q = []
                        for m in range(KK):
                            for i, (t, r) in enumerate(wsel):
                                for c in range(2):
                                    nc.tensor.matmul(
                                        ps[m][c],
                                        lhsT=wq_sb[:, t, r, :, m, :],
                                        rhs=xt[q][:, t, :, c * 512 : (c + 1) * 512],
                                        start=(i == 0),
                                        stop=(i == len(wsel) - 1),
                                        perf_mode=DR,
                                        skip_group_check=True,
                                    )
                            for c in range(2):
                                tt = tpool.tile([128, 512], bf16, tag="tt")
                                nc.scalar.activation(
                                    out=tt,
                                    in_=ps[m][c],
                                    func=mybir.ActivationFunctionType.Tanh,
                                    bias=bias_sb[:, m, b : b + 1],
                                    scale=INV_SCALE,
                                )
                                tts_q.append((m, c, tt))
                        for c in range(2):
                            n = q * 2 + c
                            tts = [tt for (m, cc, tt) in tts_q if cc == c]
                            pending.append((b, n, tts))
                        while len(pending) > 2:
                            emit_vdot(pending.pop(0))
                # flush the remaining v-dots after the loop
                for pend in pending:
                    emit_vdot(pend)
                pending = []

                # Softmax epilogue. Scores are small (|s| < ~6), so skip the
                # max subtraction: softmax = exp(s) / sum(exp(s)). The
                # per-batch sums are formed from the per-partition accum via
                # two tiny 0/1-mask matmuls (sum over n, then broadcast).
                esums = spool.tile([64, 1], f32, tag="esums")
                nc.scalar.activation(
                    out=scores64,
                    in_=scores64,
                    func=mybir.ActivationFunctionType.Exp,
                    accum_out=esums,
                )
                bsum_ps = spsum.tile([BPC, 1], f32, tag="small")
                nc.tensor.matmul(bsum_ps, lhsT=ma_sb, rhs=esums,
                                 start=True, stop=True)
                bsum_sb = spool.tile([BPC, 1], f32, tag="bsum")
                nc.vector.tensor_copy(out=bsum_sb, in_=bsum_ps)
                brep_ps = spsum.tile([64, 1], f32, tag="small")
                nc.tensor.matmul(brep_ps, lhsT=mb_sb, rhs=bsum_sb,
                                 start=True, stop=True)
                recip = spool.tile([64, 1], f32, tag="recip")
                nc.vector.reciprocal(out=recip, in_=brep_ps)
                nc.vector.tensor_scalar_mul(out=scores64, in0=scores64, scalar1=recip)
                nc.gpsimd.dma_start(
                    out=out[:, :].rearrange("b (n s) -> (b n) s", n=NCH),
                    in_=scores64,
                )

    nc.finalize()
    return nc


def prep_shared_inputs(W: np.ndarray, v: np.ndarray, decoder_hidden: np.ndarray):
    """Host-side layout marshaling of the small replicated parameters."""
    W = np.ascontiguousarray(W, dtype=np.float32)

    # fp8 weight tiles + residuals for the two encoder blocks.
    wq_tiles = np.empty((128, 2, 2, 2, 2, 128), FP8)
    for t in range(2):
        Ws = W[:, t * H : (t + 1) * H] * W_SCALE        # [h, k] scaled
        Wq = Ws.astype(FP8)
        Wr = (Ws - Wq.astype(np.float32)).astype(FP8)
        for r, Wx in ((0, Wq), (1, Wr)):
            WxT = Wx.T                                   # [k, h]
            for i in range(KK):
                for m in range(KK):
                    wq_tiles[:, t, r, i, m, :] = WxT[
                        i * 128 : (i + 1) * 128, m * 128 : (m + 1) * 128
                    ]

    # W3 tiles in f32 for the bias path.
    w3_tiles = np.empty((128, 4, 128), np.float32)
    W3T = W[:, 2 * H :].T  # [k, h]
    for kk in range(KK):
        for m in range(KK):
            w3_tiles[:, kk * 2 + m, :] = W3T[
                kk * 128 : (kk + 1) * 128, m * 128 : (m + 1) * 128
            ]

    vt = np.ascontiguousarray(v[0].reshape(KK, 128).T.astype(BF16))  # [p, kk]
    hT = decoder_hidden[0].T.astype(np.float32)  # [H, B]
    return wq_tiles, w3_tiles, vt, hT


def pack_encoder(xs: np.ndarray, xd: np.ndarray):
    """Quantize to fp8 (scaled by X_SCALE) and pack both encoder tensors into
    [B, q, p, t, i, sq] so each (b, q) slice is one contiguous transfer."""
    packed = np.empty((B, NQ, 128, 2, 2, SQ), FP8)
    for t, x in ((0, xs), (1, xd)):
        x8 = (x * np.float32(X_SCALE)).astype(FP8)        # [B, H, S]
        # [B, i, p, q, sq] -> [B, q, p, i, sq]
        v5 = x8.reshape(B, 2, 128, NQ, SQ).transpose(0, 3, 2, 1, 4)
        packed[:, :, :, t, :, :] = v5
    return packed


_CACHED = {}


def _get_nc(reps: int = 1, loop_iters: int = 0):
    key = (reps, loop_iters)
    if key not in _CACHED:
        _CACHED[key] = build_bass(reps, loop_iters)
    return _CACHED[key]


def make_in_maps(static_enc, dynamic_enc, decoder_hidden, W, v):
    wq_tiles, w3_tiles, vt, hT = prep_shared_inputs(W, v, decoder_hidden)
    static_enc = np.ascontiguousarray(static_enc, dtype=np.float32)
    dynamic_enc = np.ascontiguousarray(dynamic_enc, dtype=np.float32)
    xq_all = pack_encoder(static_enc, dynamic_enc)
    in_maps = []
    for c in range(N_CORES):
        b0 = c * BPC
        ht_c = np.ascontiguousarray(
            hT[:, b0 : b0 + BPC].reshape(KK, 128, BPC).transpose(1, 0, 2)
        )  # [p, kk, b]
        in_maps.append(
            {
                "xq": xq_all[b0 : b0 + BPC],
                "wq": wq_tiles,
                "w3": w3_tiles,
                "ht": ht_c,
                "vt": vt,
            }
        )
    return in_maps


def kernel(static_enc, dynamic_enc, decoder_hidden, W, v):
    from concourse.bass_utils import run_bass_kernel_spmd

    nc = _get_nc(reps=1)
    in_maps = make_in_maps(static_enc, dynamic_enc, decoder_hidden, W, v)
    res = run_bass_kernel_spmd(nc, in_maps, core_ids=list(range(N_CORES)))
    return np.concatenate([r["out"] for r in res.results], axis=0)


# revision 22
# speedup vs baseline: 1.5430x; 1.1263x over previous
"""Trainium2 Bass kernel for the sparse-attention scores module.

Computes, for each batch b:
    scores[b, :] = softmax_s( v . tanh(W1 @ static[b] + W2 @ dynamic[b] + W3 @ hidden[b]) )
with W = [W1 | W2 | W3] of shape [H, 3H], static/dynamic [B, H, S], hidden [B, H].

Sharding: data-parallel over B across 8 NeuronCores (8 batches per core).

The encoder streams are quantized to fp8 (e4m3) on the host, which halves the
HBM traffic vs bf16 and enables the PE's DoubleRow mode (256-deep contraction
per instruction). On TRN2 each DoubleRow matmul is bound by its 256-column
weight load (~213 ns), so instruction count is what matters: W1 is quantized
as a (Wq, Wr) pair — Wr being the fp8 quantization residual at the same
fixed-point scale, accumulated in the same PSUM group — while W2 keeps only
Wq; that is 3 DoubleRow matmuls per output tile. Measured end-to-end error
1.76e-2 rel l2 (gate 2e-2); flip RESIDUAL_BOTH for 1.38e-2 at +28 us.
Scales: x*8, W*128, undone by the ACT engine's pre-tanh scale = 1/1024.

The v-dot runs in bf16 on the PE with one-column weight loads, four chunks
at a time in 128x32 column-tiling mode so the four 512-column matmuls
overlap in different strips of the PE array, several chunks behind the tanh
producer so their semaphores are pre-satisfied. The DVE drains each strip to
SBUF and a tiny SBUF->SBUF DMA (alternating Pool/SP queues) places it at
partition 8b+n of the [64,512] scores tile, so the softmax epilogue runs on
all 64 partitions at once.
"""

import sys

sys.path.insert(0, "/opt/trn_rl_repo")

import numpy as np
import ml_dtypes

B, H, S = 64, 256, 4096
N_CORES = 8
BPC = B // N_CORES          # batches per core
KK = H // 128               # 2 contraction chunks of the output dim
NCH = S // 512              # 8 psum column chunks
NQ = 4                      # input DMA quarters along s
SQ = S // NQ                # 1024 columns per quarter

X_SCALE = 8.0
W_SCALE = 128.0
INV_SCALE = 1.0 / (X_SCALE * W_SCALE)
RESIDUAL_BOTH = False       # True: residual for W2 as well (err 1.38e-2)

FP8 = ml_dtypes.float8_e4m3
BF16 = ml_dtypes.bfloat16


def build_bass(reps: int = 1, loop_iters: int = 0):
    """Build the per-core Bass program. reps>1 unrolls the whole computation
    multiple times; loop_iters>0 additionally wraps the unrolled body in a
    hardware loop. Both are used only for timing by differencing."""
    import contextlib

    import concourse.bacc as bacc
    import concourse.tile as tile
    from concourse import mybir

    f32 = mybir.dt.float32
    f32r = mybir.dt.float32r
    bf16 = mybir.dt.bfloat16
    fp8 = mybir.dt.float8e4
    DR = mybir.MatmulPerfMode.DoubleRow

    # (tensor, use-residual) pairs for the main matmul weight sweep.
    wsel = [(0, 0), (0, 1), (1, 0)] + ([(1, 1)] if RESIDUAL_BOTH else [])

    nc = bacc.Bacc(None)

    # Encoder data, fp8, packed as [b, q, p, t, i, sq]:
    #   element = x_t[b, h = i*128 + p, s = q*SQ + sq] * X_SCALE
    # so each (b, q) transfer is fully contiguous with 4 KiB per partition.
    xq = nc.dram_tensor("xq", [BPC, NQ, 128, 2, 2, SQ], fp8, kind="ExternalInput")
    # fp8 weights [p, t, r, i, m, mc]: lhsT tiles for DoubleRow;
    # r=0 is q8(W*W_SCALE), r=1 is the quantization residual (same scale).
    wq = nc.dram_tensor("wq", [128, 2, 2, 2, 2, 128], fp8, kind="ExternalInput")
    # W3 tiles (f32) for the per-batch bias, layout [p, kk*2+m, mc].
    w3 = nc.dram_tensor("w3", [128, 4, 128], f32r, kind="ExternalInput")
    ht = nc.dram_tensor("ht", [128, KK, BPC], f32r, kind="ExternalInput")
    vt = nc.dram_tensor("vt", [128, KK], bf16, kind="ExternalInput")
    out = nc.dram_tensor("out", [BPC, S], f32, kind="ExternalOutput")

    with tile.TileContext(nc) as tc:
        with (
            tc.tile_pool(name="consts", bufs=1) as consts,
            tc.tile_pool(name="xpool", bufs=10) as xpool,
            tc.tile_pool(name="tpool", bufs=16) as tpool,
            tc.tile_pool(name="spool", bufs=2) as spool,
            tc.tile_pool(name="mpsum", bufs=4, space="PSUM") as mpsum,
            tc.tile_pool(name="vpsum", bufs=2, space="PSUM") as vpsum,
            tc.tile_pool(name="spsum", bufs=2, space="PSUM") as spsum,
        ):
            wq_sb = consts.tile([128, 2, 2, 2, 2, 128], fp8)
            nc.sync.dma_start(out=wq_sb, in_=wq[:, :, :, :, :, :])
            w3_sb = consts.tile([128, 4, 128], f32r)
            nc.sync.dma_start(out=w3_sb, in_=w3[:, :, :])
            ht_sb = consts.tile([128, KK, BPC], f32r)
            nc.sync.dma_start(out=ht_sb, in_=ht[:, :, :])
            vt_sb = consts.tile([128, KK], bf16)
            nc.sync.dma_start(out=vt_sb, in_=vt[:, :])

            # Inline 0/1 masks for the softmax normalization matmuls:
            # bsum[b] = sum_n esums[8b+n]; brep[8b+n] = bsum[b].
            ma_np = np.zeros((64, BPC), np.float32)
            mb_np = np.zeros((BPC, 64), np.float32)
            for p in range(64):
                ma_np[p, p // NCH] = 1.0
                mb_np[p // NCH, p] = 1.0
            ma_dram = nc.inline_tensor(ma_np, name="ma")
            mb_dram = nc.inline_tensor(mb_np, name="mb")
            ma_sb = consts.tile([64, BPC], f32)
            nc.sync.dma_start(out=ma_sb, in_=ma_dram[:, :])
            mb_sb = consts.tile([BPC, 64], f32)
            nc.sync.dma_start(out=mb_sb, in_=mb_dram[:, :])

            # Per-batch bias: bias[m*128+h', b] = (W3 @ hidden[b])[m*128+h']
            bias_sb = consts.tile([128, KK, BPC], f32)
            for m in range(KK):
                bias_ps = spsum.tile([128, BPC], f32, tag="small")
                for kk in range(KK):
                    nc.tensor.matmul(
                        bias_ps,
                        lhsT=w3_sb[:, kk * 2 + m, :],
                        rhs=ht_sb[:, kk, :],
                        start=(kk == 0),
                        stop=(kk == KK - 1),
                    )
                nc.vector.tensor_copy(out=bias_sb[:, m, :], in_=bias_ps)

            loop_cm = (
                tc.For_i(0, loop_iters, 1) if loop_iters else contextlib.nullcontext()
            )
            with loop_cm:
              for _ in range(reps):
                # Scores live as [64, 512] with partition p = 8*b + n so the
                # epilogue runs on all 64 partitions at once.
                scores64 = spool.tile([64, 512], f32, tag="scores")
                pending = []

                def emit_vdot4(pends):
                    # Up to four v-dot chunks run CONCURRENTLY in the PE via
                    # column tiling (128x32 mode): strip j computes into PSUM
                    # partition 32*j, so four 512-column matmuls overlap
                    # instead of serializing. They run several chunks late so
                    # the tanh results are long since ready.
                    vps = vpsum.tile([128, 512], f32, tag="vp")
                    for m in range(KK):
                        for j, (b, n, tts) in enumerate(pends):
                            nc.tensor.matmul(
                                vps[32 * j : 32 * j + 1, :],
                                lhsT=vt_sb[:, m : m + 1],
                                rhs=tts[m],
                                start=(m == 0),
                                stop=(m == KK - 1),
                                tile_position=(0, 32 * j),
                                skip_group_check=True,
                            )
                    # Compute engines may only address partition bases that
                    # are multiples of 32, so each strip drains to a staging
                    # row and a tiny SBUF->SBUF DMA places it at partition
                    # 8b+n of the scores tile. The DMAs alternate between the
                    # Pool queue (~1.5 us Q7 launch each) and the SP queue so
                    # neither becomes the bottleneck.
                    for j, (b, n, tts) in enumerate(pends):
                        row = b * NCH + n
                        stage = tpool.tile([1, 512], f32, tag="stage")
                        nc.vector.tensor_copy(
                            out=stage, in_=vps[32 * j : 32 * j + 1, :]
                        )
                        queue = nc.gpsimd if row % 2 == 0 else nc.sync
                        queue.dma_start(
                            out=scores64[row : row + 1, :],
                            in_=stage,
                        )

                for b in range(BPC):
                    # Stream the packed encoder quarters; each is one fully
                    # contiguous 512 KiB transfer (4 KiB per partition).
                    xt = []
                    for q in range(NQ):
                        xtile = xpool.tile([128, 2, 2, SQ], fp8, tag="x")
                        nc.sync.dma_start(out=xtile, in_=xq[b, q])
                        xt.append(xtile)

                    for q in range(NQ):
                        # Weight-major order: each DoubleRow weight load (the
                        # expensive part, ~213 ns for 256 columns) covers both
                        # 512-column chunks of the quarter.
                        ps = [
                            [
                                mpsum.tile([128, 512], f32, tag="ps", name=f"ps_{m}_{c}")
                                for c in range(2)
                            ]
                            for m in range(KK)
                        ]
                        tts_q = []
                        for m in range(KK):
                            for i, (t, r) in enumerate(wsel):
                                for c in range(2):
                                    nc.tensor.matmul(
                                        ps[m][c],
                                        lhsT=wq_sb[:, t, r, :, m, :],
                                        rhs=xt[q][:, t, :, c * 512 : (c + 1) * 512],
                                        start=(i == 0),
                                        stop=(i == len(wsel) - 1),
                                        perf_mode=DR,
                                        skip_group_check=True,
                                    )
                            for c in range(2):
                                tt = tpool.tile([128, 512], bf16, tag="tt")
                                nc.scalar.activation(
                                    out=tt,
                                    in_=ps[m][c],
                                    func=mybir.ActivationFunctionType.Tanh,
                                    bias=bias_sb[:, m, b : b + 1],
                                    scale=INV_SCALE,
                                )
                                tts_q.append((m, c, tt))
                        for c in range(2):
                            n = q * 2 + c
                            tts = [tt for (m, cc, tt) in tts_q if cc == c]
                            pending.append((b, n, tts))
                        if len(pending) >= 6:
                            emit_vdot4(pending[:4])
                            del pending[:4]
                # flush the remaining v-dots after the loop
                while pending:
                    emit_vdot4(pending[:4])
                    del pending[:4]

                # Softmax epilogue. Scores are small (|s| < ~6), so skip the
                # max subtraction: softmax = exp(s) / sum(exp(s)). The
                # per-batch sums are formed from the per-partition accum via
                # two tiny 0/1-mask matmuls (sum over n, then broadcast).
                esums = spool.tile([64, 1], f32, tag="esums")
                nc.scalar.activation(
                    out=scores64,
                    in_=scores64,
                    func=mybir.ActivationFunctionType.Exp,
                    accum_out=esums,
                )
                bsum_ps = spsum.tile([BPC, 1], f32, tag="small")
                nc.tensor.matmul(bsum_ps, lhsT=ma_sb, rhs=esums,
                                 start=True, stop=True)
                bsum_sb = spool.tile([BPC, 1], f32, tag="bsum")
                nc.vector.tensor_copy(out=bsum_sb, in_=bsum_ps)
                brep_ps = spsum.tile([64, 1], f32, tag="small")
                nc.tensor.matmul(brep_ps, lhsT=mb_sb, rhs=bsum_sb,
                                 start=True, stop=True)
                recip = spool.tile([64, 1], f32, tag="recip")
                nc.vector.reciprocal(out=recip, in_=brep_ps)
                nc.vector.tensor_scalar_mul(out=scores64, in0=scores64, scalar1=recip)
                nc.gpsimd.dma_start(
                    out=out[:, :].rearrange("b (n s) -> (b n) s", n=NCH),
                    in_=scores64,
                )

    nc.finalize()
    return nc


def prep_shared_inputs(W: np.ndarray, v: np.ndarray, decoder_hidden: np.ndarray):
    """Host-side layout marshaling of the small replicated parameters."""
    W = np.ascontiguousarray(W, dtype=np.float32)

    # fp8 weight tiles + residuals for the two encoder blocks.
    wq_tiles = np.empty((128, 2, 2, 2, 2, 128), FP8)
    for t in range(2):
        Ws = W[:, t * H : (t + 1) * H] * W_SCALE        # [h, k] scaled
        Wq = Ws.astype(FP8)
        Wr = (Ws - Wq.astype(np.float32)).astype(FP8)
        for r, Wx in ((0, Wq), (1, Wr)):
            WxT = Wx.T                                   # [k, h]
            for i in range(KK):
                for m in range(KK):
                    wq_tiles[:, t, r, i, m, :] = WxT[
                        i * 128 : (i + 1) * 128, m * 128 : (m + 1) * 128
                    ]

    # W3 tiles in f32 for the bias path.
    w3_tiles = np.empty((128, 4, 128), np.float32)
    W3T = W[:, 2 * H :].T  # [k, h]
    for kk in range(KK):
        for m in range(KK):
            w3_tiles[:, kk * 2 + m, :] = W3T[
                kk * 128 : (kk + 1) * 128, m * 128 : (m + 1) * 128
            ]

    vt = np.ascontiguousarray(v[0].reshape(KK, 128).T.astype(BF16))  # [p, kk]
    hT = decoder_hidden[0].T.astype(np.float32)  # [H, B]
    return wq_tiles, w3_tiles, vt, hT


def pack_encoder(xs: np.ndarray, xd: np.ndarray):
    """Quantize to fp8 (scaled by X_SCALE) and pack both encoder tensors into
    [B, q, p, t, i, sq] so each (b, q) slice is one contiguous transfer."""
    packed = np.empty((B, NQ, 128, 2, 2, SQ), FP8)
    for t, x in ((0, xs), (1, xd)):
        x8 = (x * np.float32(X_SCALE)).astype(FP8)        # [B, H, S]
        # [B, i, p, q, sq] -> [B, q, p, i, sq]
        v5 = x8.reshape(B, 2, 128, NQ, SQ).transpose(0, 3, 2, 1, 4)
        packed[:, :, :, t, :, :] = v5
    return packed


_CACHED = {}


def _get_nc(reps: int = 1, loop_iters: int = 0):
    key = (reps, loop_iters)
    if key not in _CACHED:
        _CACHED[key] = build_bass(reps, loop_iters)
    return _CACHED[key]


def make_in_maps(static_enc, dynamic_enc, decoder_hidden, W, v):
    wq_tiles, w3_tiles, vt, hT = prep_shared_inputs(W, v, decoder_hidden)
    static_enc = np.ascontiguousarray(static_enc, dtype=np.float32)
    dynamic_enc = np.ascontiguousarray(dynamic_enc, dtype=np.float32)
    xq_all = pack_encoder(static_enc, dynamic_enc)
    in_maps = []
    for c in range(N_CORES):
        b0 = c * BPC
        ht_c = np.ascontiguousarray(
            hT[:, b0 : b0 + BPC].reshape(KK, 128, BPC).transpose(1, 0, 2)
        )  # [p, kk, b]
        in_maps.append(
            {
                "xq": xq_all[b0 : b0 + BPC],
                "wq": wq_tiles,
                "w3": w3_tiles,
                "ht": ht_c,
                "vt": vt,
            }
        )
    return in_maps


def kernel(static_enc, dynamic_enc, decoder_hidden, W, v):
    from concourse.bass_utils import run_bass_kernel_spmd

    nc = _get_nc(reps=1)
    in_maps = make_in_maps(static_enc, dynamic_enc, decoder_hidden, W, v)
    res = run_bass_kernel_spmd(nc, in_maps, core_ids=list(range(N_CORES)))
    return np.concatenate([r["out"] for r in res.results], axis=0)
